# revision 28
# baseline (speedup 1.0000x reference)
"""Causal self-attention (B=2, T=2048, C=1024, H=16) on 8 trn2 NeuronCores.

Sharding: core = b*4 + g  ->  batch b, heads 4g..4g+3 (tensor-parallel on the
head/C dimension of the QKV and output projections).  Each core computes full-T
causal attention for its 4 heads and a partial output projection; the host sums
the 4 partials per batch and adds bo.

Per-core dataflow (bf16 matmuls, fp32 PSUM accum):
  All DRAM inputs are host-prepacked into the exact SBUF layout so every input
  DMA is a few large contiguous transfers (8-16KB per partition row); xt is
  n-major so the first 512-token slice lands first and compute starts ~6us in.
  QT/KT [256, T] head-major (d on partitions, two head-"pairs" of 128),
  V [T, 256] natural layout, all bf16.
  S^T = K @ Q^T per head in [T_k, T_q] blocks of [128, 512]; the two heads of
  a pair run as concurrent row-tiled matmuls (K=64, tile_position (0,0)/(64,0))
  into one [128, 1024] PSUM tile.  Diagonal blocks are column-restricted to
  the causally live strip; only one 128x128 triangle needs masking (gpsimd).
  exp: head A via ACT LUT exp (bf16 out), head B via a Schraudolph int16
  bit-trick on DVE (bitcast to bf16) - splits the exp cost across two engines.
  PV + ones-denominator: col-tiled concurrent M=64 matmul pairs accumulate
  O'^T and the replicated denominators over k; normalize via fast reciprocal
  straight from PSUM + one fused multiply (bf16 out); ypart[T, C] = O^T.T @ Wo
  in bf16, PSUM copied out alternately by ACT/DVE and DMA'd.
  S matmuls are software-pipelined one iteration ahead (across pair and block
  boundaries) so the PE never waits on the exp or normalize chains.
"""

import math

import numpy as np
import ml_dtypes

import concourse.bass as bass
import concourse.mybir as mybir
import concourse.tile as tile
from concourse import bacc
from concourse.bass_utils import run_bass_kernel_spmd
from concourse.dve_ops import RECIPROCAL_APPROX_FAST, RECIP_APPROX_FAST_CONSTS

B, T, C, H, D = 2, 2048, 1024, 16, 64
N_CORES = 8
HS = 256              # head-dim slice per core (4 heads x 64)
NQ = T // 512         # 4 q-tiles of 512
NK = T // 128         # 16 k-tiles of 128
NC8 = C // 128        # 8 contraction chunks
F32 = mybir.dt.float32
BF16 = mybir.dt.bfloat16
F8 = mybir.dt.float8e4
I16 = mybir.dt.int16

# Q/K are stored fp8e4 (unscaled) for DoubleRow S matmuls; the 1/sqrt(d)
# factor is folded into the exp evaluation instead.
SSCALE = 0.125
# Schraudolph exp in bf16 bit-space: exp(s) ~ bitcast_bf16(int16(s*A + Bc)).
SCH_A = 128.0 / math.log(2.0)
SCH_B = 16256.0 - 366393.0 / 65536.0 + 0.5

_CACHE = {}


def _build():
    nc = bacc.Bacc("TRN2", target_bir_lowering=False, debug=False,
                   num_devices=N_CORES)

    # All tensors are pre-packed host-side into SBUF layout ([128, free]).
    # xt free layout is n-major: col = 4096*n + 512*c + t'.
    xt_d = nc.dram_tensor("xt", [128, NC8 * T], BF16, kind="ExternalInput")
    wq_d = nc.dram_tensor("wq", [128, NC8 * HS], BF16, kind="ExternalInput")
    wk_d = nc.dram_tensor("wk", [128, NC8 * HS], BF16, kind="ExternalInput")
    wv_d = nc.dram_tensor("wv", [128, NC8 * HS], BF16, kind="ExternalInput")
    wo_d = nc.dram_tensor("wo", [128, 2 * C], BF16, kind="ExternalInput")
    bq_d = nc.dram_tensor("bq", [128, 2], F32, kind="ExternalInput")
    bk_d = nc.dram_tensor("bk", [128, 2], F32, kind="ExternalInput")
    bv_d = nc.dram_tensor("bv", [128, HS], F32, kind="ExternalInput")
    ms_d = nc.dram_tensor("ms", [128, 128], BF16, kind="ExternalInput")
    ob_d = nc.dram_tensor("ob", [128, 64], BF16, kind="ExternalInput")
    y_d = nc.dram_tensor("y", [T, C], F32, kind="ExternalOutput")

    def xcol(n, c, t0=0):
        return 4096 * n + 512 * c + t0

    with tile.TileContext(nc) as tc:
        with (
            tc.tile_pool(name="const", bufs=1) as cpool,
            tc.tile_pool(name="pp", bufs=4) as ppool,
            tc.tile_pool(name="onorm", bufs=4) as opool,
            tc.tile_pool(name="bc", bufs=2) as bcpool,
            tc.tile_pool(name="outp", bufs=4) as outpool,
            tc.tile_pool(name="spsum", bufs=2, space="PSUM") as spool,
            tc.tile_pool(name="opsum", bufs=1, space="PSUM") as oppool,
            tc.tile_pool(name="dpsum", bufs=1, space="PSUM") as dpool,
            tc.tile_pool(name="qkvps", bufs=1, space="PSUM") as qkvpool,
            tc.tile_pool(name="nyps", bufs=1, space="PSUM") as gpool,
        ):
            # ---- persistent SBUF tensors ----
            xt_s = cpool.tile([128, NC8 * T], BF16, tag="xt")
            wq_s = cpool.tile([128, NC8 * HS], BF16, tag="wq")
            wk_s = cpool.tile([128, NC8 * HS], BF16, tag="wk")
            wv_s = cpool.tile([128, NC8 * HS], BF16, tag="wv")
            wo_s = cpool.tile([128, 2 * C], BF16, tag="wo")
            v_s = cpool.tile([128, NK * HS], BF16, tag="vs")
            qt_s = [cpool.tile([128, T], F8, tag=f"qt{p}", name=f"qt{p}")
                    for p in range(2)]
            kt_s = [cpool.tile([128, T], F8, tag=f"kt{p}", name=f"kt{p}")
                    for p in range(2)]
            # d-pair-packed copies for DoubleRow: [64, (i, T)] where
            # partition pp<32 holds head A's d-rows (2pp, 2pp+1), pp>=32
            # head B's; free dim i in {0,1} selects the pair element.
            qtp_s = [cpool.tile([64, 2 * T], F8, tag=f"qtp{p}",
                                name=f"qtp{p}") for p in range(2)]
            ktp_s = [cpool.tile([64, 2 * T], F8, tag=f"ktp{p}",
                                name=f"ktp{p}") for p in range(2)]
            ms_s = cpool.tile([128, 128], BF16, tag="ms")
            bq_s = cpool.tile([128, 2], F32, tag="bq")
            bk_s = cpool.tile([128, 2], F32, tag="bk")
            bv_s = cpool.tile([128, HS], F32, tag="bv")
            ob_s = cpool.tile([128, 64], BF16, tag="ob")

            # ---- PE warm-up: ~3.5us of tiny matmuls on a zeroed tile so the
            # HAM clock-gate releases before the first real chain arrives ----
            wu_s = cpool.tile([128, 64], BF16, tag="wu")
            nc.vector.memset(wu_s[:], 0.0)
            for _ in range(24):
                wu_ps = gpool.tile([128, 512], F32, tag="g")
                nc.tensor.matmul(wu_ps[0:64, 0:64], wu_s[:], wu_s[:],
                                 start=True, stop=True)

            # ---- input DMAs, dependency-priority order, all contiguous;
            # the first chains' inputs are split fine-grained so the c-chunk
            # matmuls can start as soon as their own slice lands ----
            nc.sync.dma_start(out=wq_s[:], in_=wq_d.ap())
            nc.sync.dma_start(out=xt_s[:, 0:2048], in_=xt_d.ap()[:, 0:2048])
            nc.sync.dma_start(out=xt_s[:, 2048:4096],
                              in_=xt_d.ap()[:, 2048:4096])
            nc.sync.dma_start(out=wk_s[:], in_=wk_d.ap())
            nc.sync.dma_start(out=wv_s[:], in_=wv_d.ap())
            nc.sync.dma_start(out=bq_s[:], in_=bq_d.ap())
            nc.sync.dma_start(out=bk_s[:], in_=bk_d.ap())
            nc.sync.dma_start(out=bv_s[:], in_=bv_d.ap())
            nc.sync.dma_start(out=ms_s[:], in_=ms_d.ap())
            nc.sync.dma_start(out=ob_s[:], in_=ob_d.ap())
            nc.sync.dma_start(out=xt_s[:, 4096:8192],
                              in_=xt_d.ap()[:, 4096:8192])

            def qkv_group_thunks(n):
                """Per-group emission thunks for QT/KT/V of q/k-range n.
                Q/K are written fp8 and then DMA-repacked into the d-pair
                layout the DoubleRow S matmuls consume."""
                thunks = []
                for w_s, b_s, t_s, tp_s in (
                        (wq_s, bq_s, qt_s, qtp_s), (wk_s, bk_s, kt_s, ktp_s)):
                    for p in range(2):
                        def th(p=p, w_s=w_s, b_s=b_s, t_s=t_s, tp_s=tp_s):
                            ps = qkvpool.tile([128, 512], F32, tag="qg")
                            for c in range(NC8):
                                nc.tensor.matmul(
                                    ps[:],
                                    w_s[:, HS * c + 128 * p:
                                        HS * c + 128 * (p + 1)],
                                    xt_s[:, xcol(n, c):xcol(n, c) + 512],
                                    start=(c == 0), stop=(c == NC8 - 1))
                            nc.scalar.add(
                                t_s[p][:, 512 * n:512 * (n + 1)], ps[:],
                                b_s[:, p:p + 1])
                            for e in range(2):
                                tp3 = tp_s[p][:].rearrange(
                                    "p (i t) -> p i t", i=2)
                                nc.sync.dma_start(
                                    out=tp3[32 * e:32 * (e + 1), :,
                                            512 * n:512 * (n + 1)],
                                    in_=t_s[p][64 * e:64 * (e + 1),
                                               512 * n:512 * (n + 1)])
                        thunks.append(th)
                for u in range(4):
                    def th(u=u):
                        t_idx = 4 * n + u
                        ps = qkvpool.tile([128, HS], F32, tag="qg")
                        for c in range(NC8):
                            nc.tensor.matmul(
                                ps[:],
                                xt_s[:, xcol(n, c, 128 * u):
                                     xcol(n, c, 128 * (u + 1))],
                                wv_s[:, HS * c:HS * (c + 1)],
                                start=(c == 0), stop=(c == NC8 - 1))
                        nc.vector.tensor_add(
                            out=v_s[:, HS * t_idx:HS * (t_idx + 1)],
                            in0=ps[:], in1=bv_s[:])
                    thunks.append(th)
                return thunks

            def proj_group_thunks(j, onorm, split_copy=False):
                thunks = []
                for u in range(4):
                    for n2 in range(2):
                        def th(u=u, n2=n2):
                            # alternate the PSUM bank (and the copy engine) so
                            # consecutive thunks pipeline instead of
                            # serializing on one bank
                            pool, tag = ((gpool, "g") if (2 * u + n2) % 2 == 0
                                         else (qkvpool, "qg"))
                            y_ps = pool.tile([128, 512], F32, tag=tag)
                            for p in range(2):
                                nc.tensor.matmul(
                                    y_ps[:],
                                    onorm[p][:, 128 * u:128 * (u + 1)],
                                    wo_s[:, C * p + 512 * n2:
                                         C * p + 512 * (n2 + 1)],
                                    start=(p == 0), stop=(p == 1))
                            out_t = outpool.tile([128, 512], F32, tag="out")
                            if split_copy:
                                nc.scalar.copy(out_t[:, 0:256],
                                               y_ps[:, 0:256])
                                nc.vector.tensor_copy(out_t[:, 256:512],
                                                      y_ps[:, 256:512])
                            elif (u + n2) % 2 == 0:
                                nc.scalar.copy(out_t[:], y_ps[:])
                            else:
                                nc.vector.tensor_copy(out_t[:], y_ps[:])
                            nc.sync.dma_start(
                                out=y_d.ap()[512 * j + 128 * u:
                                             512 * j + 128 * (u + 1),
                                             512 * n2:512 * (n2 + 1)],
                                in_=out_t[:])
                        thunks.append(th)
                return thunks

            filler = []

            def emit_filler(nmax):
                for _ in range(min(nmax, len(filler))):
                    filler.pop(0)()

            def emit_s(j, p, k):
                """S^T block for q-tile j, pair p, k-tile k (col-restricted
                on diagonal blocks), fp8 DoubleRow. Returns (s_ps, off)."""
                m = k - 4 * j
                off = 128 * m if m > 0 else 0
                s_ps = spool.tile([128, 1024], F32, tag="s",
                                  name=f"s_{j}_{p}_{k}")
                ktp3 = ktp_s[p][:].rearrange("p (i t) -> p i t", i=2)
                qtp3 = qtp_s[p][:].rearrange("p (i t) -> p i t", i=2)
                for e in range(2):
                    nc.tensor.matmul(
                        s_ps[:, 512 * e + off:512 * (e + 1)],
                        ktp3[32 * e:32 * (e + 1), :, 128 * k:128 * (k + 1)],
                        qtp3[32 * e:32 * (e + 1), :,
                             512 * j + off:512 * (j + 1)],
                        start=True, stop=True,
                        perf_mode=mybir.MatmulPerfMode.DoubleRow,
                        tile_position=(32 * e, 0))
                return s_ps, off

            # Flat (j, pair) schedule with S pipelined one step ahead,
            # including across pair/block boundaries.
            pairs = [(j, p) for j in range(NQ) for p in range(2)]

            def next_pair(idx):
                return pairs[idx + 1] if idx + 1 < len(pairs) else None

            for th in qkv_group_thunks(0):
                th()
            # bulk DMAs issued after group 0 so their SP dispatch doesn't
            # delay the group-0 repacks the first attention block needs
            for n in (2, 3):
                nc.sync.dma_start(out=xt_s[:, 4096 * n:4096 * (n + 1)],
                                  in_=xt_d.ap()[:, 4096 * n:4096 * (n + 1)])
            nc.sync.dma_start(out=wo_s[:], in_=wo_d.ap())

            s_cur = emit_s(0, 0, 3)
            onorm_cur = []
            for idx, (j, p) in enumerate(pairs):
                if p == 0:
                    filler.extend(
                        qkv_group_thunks(j + 1) if j + 1 < NQ else [])
                    onorm_cur = []
                niter = 8 * (j + 1)
                per_iter = -(-len(filler) // max(1, niter - 4))
                nk = 4 * (j + 1)
                o_ps = oppool.tile([128, 512], F32, tag="o",
                                   name=f"o_ps{p}_{j}")
                dn_ps = dpool.tile([128, 512], F32, tag="dn",
                                   name=f"dn_{j}_{p}")
                # diagonal (masked, column-restricted) tiles first, narrowest
                # first: their long S->exp->mask->PV latency chains hide under
                # the previous pair's drain, and the block ends on full-width
                # streaming tiles.  PSUM first-touch-overwrites make the
                # restricted-width start legal.
                k_order = list(range(nk - 1, 4 * j - 1, -1)) + \
                    list(range(0, 4 * j))
                for ki, k in enumerate(k_order):
                    m = k - 4 * j
                    s_ps, off = s_cur
                    pt = ppool.tile([128, 1024], BF16, tag="p",
                                    name=f"p_{j}_{k}_{p}")
                    # exp: head A on ACT, head B on DVE (bit-trick). On the
                    # pair's first iteration ACT takes both halves: the DVE
                    # is busy with the previous pair's normalize, which gates
                    # this pair's PV/denominator via the PSUM bank reuse.
                    if ki == 0:
                        nc.scalar.activation(
                            pt[:, off:1024], s_ps[:, off:1024],
                            mybir.ActivationFunctionType.Exp, scale=SSCALE)
                    else:
                        nc.scalar.activation(
                            pt[:, off:512], s_ps[:, off:512],
                            mybir.ActivationFunctionType.Exp, scale=SSCALE)
                        nc.vector.tensor_scalar(
                            out=pt[:, 512 + off:1024].bitcast(I16),
                            in0=s_ps[:, 512 + off:1024],
                            scalar1=SCH_A * SSCALE, scalar2=SCH_B,
                            op0=mybir.AluOpType.mult,
                            op1=mybir.AluOpType.add)
                    if m >= 0:
                        # head A strip on gpsimd; head B strip on DVE (same
                        # engine as its exp -> pipelined, no cross-engine sync)
                        sl0 = slice(128 * m, 128 * (m + 1))
                        nc.gpsimd.tensor_mul(
                            out=pt[:, sl0], in0=pt[:, sl0], in1=ms_s[:])
                        sl1 = slice(512 + 128 * m, 512 + 128 * (m + 1))
                        nc.vector.tensor_mul(
                            out=pt[:, sl1], in0=pt[:, sl1], in1=ms_s[:])
                    # pipeline the next S block ahead of this iteration's
                    # PV/denominator so the PE never idles on the exp chain
                    if ki + 1 < nk:
                        s_cur = emit_s(j, p, k_order[ki + 1])
                    else:
                        np_ = next_pair(idx)
                        if np_ is not None:
                            nj, npr = np_
                            s_cur = emit_s(nj, npr, 4 * (nj + 1) - 1)
                    for e in range(2):
                        nc.tensor.matmul(
                            dn_ps[64 * e:64 * (e + 1), off:512],
                            ob_s[:],
                            pt[:, 512 * e + off:512 * (e + 1)],
                            start=(ki == 0), stop=(ki == nk - 1),
                            tile_position=(0, 64 * e))
                    for e in range(2):
                        nc.tensor.matmul(
                            o_ps[64 * e:64 * (e + 1), off:512],
                            v_s[:, HS * k + 128 * p + 64 * e:
                                HS * k + 128 * p + 64 * (e + 1)],
                            pt[:, 512 * e + off:512 * (e + 1)],
                            start=(ki == 0), stop=(ki == nk - 1),
                            tile_position=(0, 64 * e))
                    emit_filler(per_iter)
                # normalize this pair:  O = O' * (1/denom), bf16 out
                rr = bcpool.tile([128, 512], F32, tag="rr",
                                 name=f"rr_{j}_{p}")
                nc.vector._custom_dve(
                    RECIPROCAL_APPROX_FAST, out=rr[:], in0=dn_ps[:],
                    s0=RECIP_APPROX_FAST_CONSTS["s0"],
                    s1=RECIP_APPROX_FAST_CONSTS["s1"],
                    imm2=RECIP_APPROX_FAST_CONSTS["imm2"])
                ot = opool.tile([128, 512], BF16, tag="onorm",
                                name=f"onorm_{j}_{p}")
                nc.vector.tensor_mul(out=ot[:], in0=o_ps[:], in1=rr[:])
                onorm_cur.append(ot)
                if p == 1:
                    filler.extend(proj_group_thunks(
                        j, onorm_cur, split_copy=(j == NQ - 1)))
            emit_filler(len(filler))

    nc.compile()
    return nc


def _get_nc():
    if "nc" not in _CACHE:
        _CACHE["nc"] = _build()
    return _CACHE["nc"]


def _mask():
    # [128, 128] lower-triangle-of-block mask: col c valid for row r iff
    # c >= r (applies identically to every diagonal 128-strip).
    k = np.arange(128)[:, None]
    q = np.arange(128)[None, :]
    return (q >= k).astype(np.float32).astype(ml_dtypes.bfloat16)


def _pack_w(w):
    # [1024, M] -> SBUF layout [128, 8*M]: col = M*c + m for C-chunk c
    m = w.shape[1]
    return np.ascontiguousarray(
        w.reshape(8, 128, m).transpose(1, 0, 2).reshape(128, 8 * m))


def kernel(x, Wq, bq, Wk, bk, Wv, bv, Wo, bo):
    x = np.asarray(x, np.float32)
    Wq, bq = np.asarray(Wq, np.float32), np.asarray(bq, np.float32)
    Wk, bk = np.asarray(Wk, np.float32), np.asarray(bk, np.float32)
    Wv, bv = np.asarray(Wv, np.float32), np.asarray(bv, np.float32)
    Wo, bo = np.asarray(Wo, np.float32), np.asarray(bo, np.float32)

    nc = _get_nc()
    ms = _mask()
    ones_bf = np.ones((128, 64), np.float32).astype(ml_dtypes.bfloat16)

    in_maps = []
    for core in range(N_CORES):
        b, g = divmod(core, 4)
        sl = slice(HS * g, HS * (g + 1))
        # xt n-major: [128, (n, c, 512)]; xt[r, 4096n+512c+t'] = x[b][512n+t', 128c+r]
        xt = np.ascontiguousarray(
            x[b].reshape(4, 512, 8, 128).transpose(3, 0, 2, 1)
            .reshape(128, 16384))
        # wo: [256, 1024] -> [128, (p, 1024)]
        wo = np.ascontiguousarray(
            Wo[sl, :].reshape(2, 128, 1024).transpose(1, 0, 2)
            .reshape(128, 2048))
        in_maps.append({
            "xt": xt.astype(ml_dtypes.bfloat16),
            "wq": _pack_w(Wq[:, sl]).astype(ml_dtypes.bfloat16),
            "wk": _pack_w(Wk[:, sl]).astype(ml_dtypes.bfloat16),
            "wv": _pack_w(Wv[:, sl]).astype(ml_dtypes.bfloat16),
            "wo": wo.astype(ml_dtypes.bfloat16),
            "bq": bq[sl].reshape(2, 128).T.copy(),
            "bk": bk[sl].reshape(2, 128).T.copy(),
            "bv": np.broadcast_to(bv[sl], (128, HS)).copy(),
            "ms": ms,
            "ob": ones_bf,
        })

    res = run_bass_kernel_spmd(nc, in_maps, core_ids=list(range(N_CORES)),
                               **_CACHE.get("run_kwargs", {}))
    _CACHE["last_result"] = res

    y = np.zeros((B, T, C), np.float32)
    for core in range(N_CORES):
        b = core // 4
        y[b] += res.results[core]["y"]
    y += bo
    return y


# revision 30
# speedup vs baseline: 1.0187x; 1.0187x over previous
"""Causal self-attention (B=2, T=2048, C=1024, H=16) on 8 trn2 NeuronCores.

Sharding: core = b*4 + g  ->  batch b, heads 4g..4g+3 (tensor-parallel on the
head/C dimension of the QKV and output projections).  Each core computes full-T
causal attention for its 4 heads and a partial output projection; the host sums
the 4 partials per batch and adds bo.

Per-core dataflow (bf16 matmuls, fp32 PSUM accum):
  All DRAM inputs are host-prepacked into the exact SBUF layout so every input
  DMA is a few large contiguous transfers (8-16KB per partition row); xt is
  n-major so the first 512-token slice lands first and compute starts ~6us in.
  QT/KT [256, T] head-major (d on partitions, two head-"pairs" of 128),
  V [T, 256] natural layout, all bf16.
  S^T = K @ Q^T per head in [T_k, T_q] blocks of [128, 512]; the two heads of
  a pair run as concurrent row-tiled matmuls (K=64, tile_position (0,0)/(64,0))
  into one [128, 1024] PSUM tile.  Diagonal blocks are column-restricted to
  the causally live strip; only one 128x128 triangle needs masking (gpsimd).
  exp: head A via ACT LUT exp (bf16 out), head B via a Schraudolph int16
  bit-trick on DVE (bitcast to bf16) - splits the exp cost across two engines.
  PV + ones-denominator: col-tiled concurrent M=64 matmul pairs accumulate
  O'^T and the replicated denominators over k; normalize via fast reciprocal
  straight from PSUM + one fused multiply (bf16 out); ypart[T, C] = O^T.T @ Wo
  in bf16, PSUM copied out alternately by ACT/DVE and DMA'd.
  S matmuls are software-pipelined one iteration ahead (across pair and block
  boundaries) so the PE never waits on the exp or normalize chains.
"""

import math

import numpy as np
import ml_dtypes

import concourse.bass as bass
import concourse.mybir as mybir
import concourse.tile as tile
from concourse import bacc
from concourse.bass_utils import run_bass_kernel_spmd
from concourse.dve_ops import RECIPROCAL_APPROX_FAST, RECIP_APPROX_FAST_CONSTS

B, T, C, H, D = 2, 2048, 1024, 16, 64
N_CORES = 8
HS = 256              # head-dim slice per core (4 heads x 64)
NQ = T // 512         # 4 q-tiles of 512
NK = T // 128         # 16 k-tiles of 128
NC8 = C // 128        # 8 contraction chunks
F32 = mybir.dt.float32
BF16 = mybir.dt.bfloat16
I16 = mybir.dt.int16

# Schraudolph exp in bf16 bit-space: exp(s) ~ bitcast_bf16(int16(s*A + Bc)).
SCH_A = 128.0 / math.log(2.0)
SCH_B = 16256.0 - 366393.0 / 65536.0 + 0.5

_CACHE = {}


def _build():
    nc = bacc.Bacc("TRN2", target_bir_lowering=False, debug=False,
                   num_devices=N_CORES)

    # All tensors are pre-packed host-side into SBUF layout ([128, free]).
    # xt free layout is n-major: col = 4096*n + 512*c + t'.
    xt_d = nc.dram_tensor("xt", [128, NC8 * T], BF16, kind="ExternalInput")
    wq_d = nc.dram_tensor("wq", [128, NC8 * HS], BF16, kind="ExternalInput")
    wk_d = nc.dram_tensor("wk", [128, NC8 * HS], BF16, kind="ExternalInput")
    wv_d = nc.dram_tensor("wv", [128, NC8 * HS], BF16, kind="ExternalInput")
    wo_d = nc.dram_tensor("wo", [128, 2 * C], BF16, kind="ExternalInput")
    bq_d = nc.dram_tensor("bq", [128, 2], F32, kind="ExternalInput")
    bk_d = nc.dram_tensor("bk", [128, 2], F32, kind="ExternalInput")
    bv_d = nc.dram_tensor("bv", [128, HS], F32, kind="ExternalInput")
    ms_d = nc.dram_tensor("ms", [128, 128], BF16, kind="ExternalInput")
    ob_d = nc.dram_tensor("ob", [128, 64], BF16, kind="ExternalInput")
    y_d = nc.dram_tensor("y", [T, C], F32, kind="ExternalOutput")

    def xcol(n, c, t0=0):
        return 4096 * n + 512 * c + t0

    with tile.TileContext(nc) as tc:
        with (
            tc.tile_pool(name="const", bufs=1) as cpool,
            tc.tile_pool(name="pp", bufs=4) as ppool,
            tc.tile_pool(name="onorm", bufs=4) as opool,
            tc.tile_pool(name="bc", bufs=2) as bcpool,
            tc.tile_pool(name="outp", bufs=4) as outpool,
            tc.tile_pool(name="spsum", bufs=2, space="PSUM") as spool,
            tc.tile_pool(name="opsum", bufs=1, space="PSUM") as oppool,
            tc.tile_pool(name="dpsum", bufs=1, space="PSUM") as dpool,
            tc.tile_pool(name="qkvps", bufs=1, space="PSUM") as qkvpool,
            tc.tile_pool(name="nyps", bufs=1, space="PSUM") as gpool,
        ):
            # ---- persistent SBUF tensors ----
            xt_s = cpool.tile([128, NC8 * T], BF16, tag="xt")
            wq_s = cpool.tile([128, NC8 * HS], BF16, tag="wq")
            wk_s = cpool.tile([128, NC8 * HS], BF16, tag="wk")
            wv_s = cpool.tile([128, NC8 * HS], BF16, tag="wv")
            wo_s = cpool.tile([128, 2 * C], BF16, tag="wo")
            v_s = cpool.tile([128, NK * HS], BF16, tag="vs")
            qt_s = [cpool.tile([128, T], BF16, tag=f"qt{p}", name=f"qt{p}")
                    for p in range(2)]
            kt_s = [cpool.tile([128, T], BF16, tag=f"kt{p}", name=f"kt{p}")
                    for p in range(2)]
            ms_s = cpool.tile([128, 128], BF16, tag="ms")
            bq_s = cpool.tile([128, 2], F32, tag="bq")
            bk_s = cpool.tile([128, 2], F32, tag="bk")
            bv_s = cpool.tile([128, HS], F32, tag="bv")
            ob_s = cpool.tile([128, 64], BF16, tag="ob")

            # ---- PE warm-up: ~3.5us of tiny matmuls on a zeroed tile so the
            # HAM clock-gate releases before the first real chain arrives ----
            wu_s = cpool.tile([128, 64], BF16, tag="wu")
            nc.vector.memset(wu_s[:], 0.0)
            for _ in range(24):
                wu_ps = gpool.tile([128, 512], F32, tag="g")
                nc.tensor.matmul(wu_ps[0:64, 0:64], wu_s[:], wu_s[:],
                                 start=True, stop=True)

            # ---- input DMAs, dependency-priority order, all contiguous;
            # the first chains' inputs are split fine-grained so the c-chunk
            # matmuls can start as soon as their own slice lands ----
            nc.sync.dma_start(out=wq_s[:], in_=wq_d.ap())
            nc.sync.dma_start(out=xt_s[:, 0:2048], in_=xt_d.ap()[:, 0:2048])
            nc.sync.dma_start(out=xt_s[:, 2048:4096],
                              in_=xt_d.ap()[:, 2048:4096])
            nc.sync.dma_start(out=wk_s[:], in_=wk_d.ap())
            nc.sync.dma_start(out=wv_s[:], in_=wv_d.ap())
            nc.sync.dma_start(out=bq_s[:], in_=bq_d.ap())
            nc.sync.dma_start(out=bk_s[:], in_=bk_d.ap())
            nc.sync.dma_start(out=bv_s[:], in_=bv_d.ap())
            nc.sync.dma_start(out=ms_s[:], in_=ms_d.ap())
            nc.sync.dma_start(out=ob_s[:], in_=ob_d.ap())
            for n in range(1, NQ):
                nc.sync.dma_start(out=xt_s[:, 4096 * n:4096 * (n + 1)],
                                  in_=xt_d.ap()[:, 4096 * n:4096 * (n + 1)])
            nc.sync.dma_start(out=wo_s[:], in_=wo_d.ap())

            def qkv_group_thunks(n):
                """Per-group emission thunks for QT/KT/V of q/k-range n."""
                thunks = []
                for w_s, b_s, t_s in ((wq_s, bq_s, qt_s), (wk_s, bk_s, kt_s)):
                    for p in range(2):
                        def th(p=p, w_s=w_s, b_s=b_s, t_s=t_s):
                            ps = qkvpool.tile([128, 512], F32, tag="qg")
                            for c in range(NC8):
                                nc.tensor.matmul(
                                    ps[:],
                                    w_s[:, HS * c + 128 * p:
                                        HS * c + 128 * (p + 1)],
                                    xt_s[:, xcol(n, c):xcol(n, c) + 512],
                                    start=(c == 0), stop=(c == NC8 - 1))
                            nc.scalar.add(
                                t_s[p][:, 512 * n:512 * (n + 1)], ps[:],
                                b_s[:, p:p + 1])
                        thunks.append(th)
                for u in range(4):
                    def th(u=u):
                        t_idx = 4 * n + u
                        ps = qkvpool.tile([128, HS], F32, tag="qg")
                        for c in range(NC8):
                            nc.tensor.matmul(
                                ps[:],
                                xt_s[:, xcol(n, c, 128 * u):
                                     xcol(n, c, 128 * (u + 1))],
                                wv_s[:, HS * c:HS * (c + 1)],
                                start=(c == 0), stop=(c == NC8 - 1))
                        nc.vector.tensor_add(
                            out=v_s[:, HS * t_idx:HS * (t_idx + 1)],
                            in0=ps[:], in1=bv_s[:])
                    thunks.append(th)
                return thunks

            def proj_group_thunks(j, onorm, split_copy=False):
                thunks = []
                for u in range(4):
                    for n2 in range(2):
                        def th(u=u, n2=n2):
                            # alternate the PSUM bank (and the copy engine) so
                            # consecutive thunks pipeline instead of
                            # serializing on one bank
                            pool, tag = ((gpool, "g") if (2 * u + n2) % 2 == 0
                                         else (qkvpool, "qg"))
                            y_ps = pool.tile([128, 512], F32, tag=tag)
                            for p in range(2):
                                nc.tensor.matmul(
                                    y_ps[:],
                                    onorm[p][:, 128 * u:128 * (u + 1)],
                                    wo_s[:, C * p + 512 * n2:
                                         C * p + 512 * (n2 + 1)],
                                    start=(p == 0), stop=(p == 1))
                            out_t = outpool.tile([128, 512], F32, tag="out")
                            if split_copy:
                                nc.scalar.copy(out_t[:, 0:256],
                                               y_ps[:, 0:256])
                                nc.vector.tensor_copy(out_t[:, 256:512],
                                                      y_ps[:, 256:512])
                            elif (u + n2) % 2 == 0:
                                nc.scalar.copy(out_t[:], y_ps[:])
                            else:
                                nc.vector.tensor_copy(out_t[:], y_ps[:])
                            nc.sync.dma_start(
                                out=y_d.ap()[512 * j + 128 * u:
                                             512 * j + 128 * (u + 1),
                                             512 * n2:512 * (n2 + 1)],
                                in_=out_t[:])
                        thunks.append(th)
                return thunks

            filler = []

            def emit_filler(nmax):
                for _ in range(min(nmax, len(filler))):
                    filler.pop(0)()

            def emit_s(j, p, k):
                """S^T block for q-tile j, pair p, k-tile k (col-restricted
                on diagonal blocks). Returns (s_ps, off)."""
                m = k - 4 * j
                off = 128 * m if m > 0 else 0
                s_ps = spool.tile([128, 1024], F32, tag="s",
                                  name=f"s_{j}_{p}_{k}")
                for e in range(2):
                    nc.tensor.matmul(
                        s_ps[:, 512 * e + off:512 * (e + 1)],
                        kt_s[p][64 * e:64 * (e + 1), 128 * k:128 * (k + 1)],
                        qt_s[p][64 * e:64 * (e + 1),
                                512 * j + off:512 * (j + 1)],
                        start=True, stop=True,
                        tile_position=(64 * e, 0))
                return s_ps, off

            # Flat (j, pair) schedule with S pipelined one step ahead,
            # including across pair/block boundaries.
            pairs = [(j, p) for j in range(NQ) for p in range(2)]

            def next_pair(idx):
                return pairs[idx + 1] if idx + 1 < len(pairs) else None

            for th in qkv_group_thunks(0):
                th()

            s_cur = emit_s(0, 0, 3)
            onorm_cur = []
            for idx, (j, p) in enumerate(pairs):
                if p == 0:
                    filler.extend(
                        qkv_group_thunks(j + 1) if j + 1 < NQ else [])
                    onorm_cur = []
                niter = 8 * (j + 1)
                per_iter = -(-len(filler) // max(1, niter - 2))
                nk = 4 * (j + 1)
                o_ps = oppool.tile([128, 512], F32, tag="o",
                                   name=f"o_ps{p}_{j}")
                dn_ps = dpool.tile([128, 512], F32, tag="dn",
                                   name=f"dn_{j}_{p}")
                # diagonal (masked, column-restricted) tiles first, narrowest
                # first: their long S->exp->mask->PV latency chains hide under
                # the previous pair's drain, and the block ends on full-width
                # streaming tiles.  PSUM first-touch-overwrites make the
                # restricted-width start legal.
                k_order = list(range(nk - 1, 4 * j - 1, -1)) + \
                    list(range(0, 4 * j))
                for ki, k in enumerate(k_order):
                    m = k - 4 * j
                    s_ps, off = s_cur
                    pt = ppool.tile([128, 1024], BF16, tag="p",
                                    name=f"p_{j}_{k}_{p}")
                    # exp: head A on ACT, head B on DVE (bit-trick). On the
                    # pair's first iteration ACT takes both halves: the DVE
                    # is busy with the previous pair's normalize, which gates
                    # this pair's PV/denominator via the PSUM bank reuse.
                    if ki == 0:
                        nc.scalar.activation(
                            pt[:, off:1024], s_ps[:, off:1024],
                            mybir.ActivationFunctionType.Exp)
                    else:
                        nc.scalar.activation(
                            pt[:, off:512], s_ps[:, off:512],
                            mybir.ActivationFunctionType.Exp)
                        nc.vector.tensor_scalar(
                            out=pt[:, 512 + off:1024].bitcast(I16),
                            in0=s_ps[:, 512 + off:1024],
                            scalar1=SCH_A, scalar2=SCH_B,
                            op0=mybir.AluOpType.mult,
                            op1=mybir.AluOpType.add)
                    if m >= 0:
                        # head A strip on gpsimd; head B strip on DVE (same
                        # engine as its exp -> pipelined, no cross-engine sync)
                        sl0 = slice(128 * m, 128 * (m + 1))
                        nc.gpsimd.tensor_mul(
                            out=pt[:, sl0], in0=pt[:, sl0], in1=ms_s[:])
                        sl1 = slice(512 + 128 * m, 512 + 128 * (m + 1))
                        nc.vector.tensor_mul(
                            out=pt[:, sl1], in0=pt[:, sl1], in1=ms_s[:])
                    # pipeline the next S block ahead of this iteration's
                    # PV/denominator so the PE never idles on the exp chain
                    if ki + 1 < nk:
                        s_cur = emit_s(j, p, k_order[ki + 1])
                    else:
                        np_ = next_pair(idx)
                        if np_ is not None:
                            nj, npr = np_
                            s_cur = emit_s(nj, npr, 4 * (nj + 1) - 1)
                    for e in range(2):
                        nc.tensor.matmul(
                            dn_ps[64 * e:64 * (e + 1), off:512],
                            ob_s[:],
                            pt[:, 512 * e + off:512 * (e + 1)],
                            start=(ki == 0), stop=(ki == nk - 1),
                            tile_position=(0, 64 * e))
                    for e in range(2):
                        nc.tensor.matmul(
                            o_ps[64 * e:64 * (e + 1), off:512],
                            v_s[:, HS * k + 128 * p + 64 * e:
                                HS * k + 128 * p + 64 * (e + 1)],
                            pt[:, 512 * e + off:512 * (e + 1)],
                            start=(ki == 0), stop=(ki == nk - 1),
                            tile_position=(0, 64 * e))
                    emit_filler(per_iter)
                # normalize this pair:  O = O' * (1/denom), bf16 out
                rr = bcpool.tile([128, 512], F32, tag="rr",
                                 name=f"rr_{j}_{p}")
                nc.vector._custom_dve(
                    RECIPROCAL_APPROX_FAST, out=rr[:], in0=dn_ps[:],
                    s0=RECIP_APPROX_FAST_CONSTS["s0"],
                    s1=RECIP_APPROX_FAST_CONSTS["s1"],
                    imm2=RECIP_APPROX_FAST_CONSTS["imm2"])
                ot = opool.tile([128, 512], BF16, tag="onorm",
                                name=f"onorm_{j}_{p}")
                nc.vector.tensor_mul(out=ot[:], in0=o_ps[:], in1=rr[:])
                onorm_cur.append(ot)
                if p == 1:
                    filler.extend(proj_group_thunks(
                        j, onorm_cur, split_copy=(j == NQ - 1)))
            emit_filler(len(filler))

    nc.compile()
    return nc


def _get_nc():
    if "nc" not in _CACHE:
        _CACHE["nc"] = _build()
    return _CACHE["nc"]


def _mask():
    # [128, 128] lower-triangle-of-block mask: col c valid for row r iff
    # c >= r (applies identically to every diagonal 128-strip).
    k = np.arange(128)[:, None]
    q = np.arange(128)[None, :]
    return (q >= k).astype(np.float32).astype(ml_dtypes.bfloat16)


def _pack_w(w):
    # [1024, M] -> SBUF layout [128, 8*M]: col = M*c + m for C-chunk c
    m = w.shape[1]
    return np.ascontiguousarray(
        w.reshape(8, 128, m).transpose(1, 0, 2).reshape(128, 8 * m))


def kernel(x, Wq, bq, Wk, bk, Wv, bv, Wo, bo):
    x = np.asarray(x, np.float32)
    Wq, bq = np.asarray(Wq, np.float32), np.asarray(bq, np.float32)
    Wk, bk = np.asarray(Wk, np.float32), np.asarray(bk, np.float32)
    Wv, bv = np.asarray(Wv, np.float32), np.asarray(bv, np.float32)
    Wo, bo = np.asarray(Wo, np.float32), np.asarray(bo, np.float32)

    nc = _get_nc()
    ms = _mask()
    ones_bf = np.ones((128, 64), np.float32).astype(ml_dtypes.bfloat16)

    in_maps = []
    for core in range(N_CORES):
        b, g = divmod(core, 4)
        sl = slice(HS * g, HS * (g + 1))
        # xt n-major: [128, (n, c, 512)]; xt[r, 4096n+512c+t'] = x[b][512n+t', 128c+r]
        xt = np.ascontiguousarray(
            x[b].reshape(4, 512, 8, 128).transpose(3, 0, 2, 1)
            .reshape(128, 16384))
        # wo: [256, 1024] -> [128, (p, 1024)]
        wo = np.ascontiguousarray(
            Wo[sl, :].reshape(2, 128, 1024).transpose(1, 0, 2)
            .reshape(128, 2048))
        in_maps.append({
            "xt": xt.astype(ml_dtypes.bfloat16),
            "wq": _pack_w(Wq[:, sl] * 0.125).astype(ml_dtypes.bfloat16),
            "wk": _pack_w(Wk[:, sl]).astype(ml_dtypes.bfloat16),
            "wv": _pack_w(Wv[:, sl]).astype(ml_dtypes.bfloat16),
            "wo": wo.astype(ml_dtypes.bfloat16),
            "bq": (bq[sl] * 0.125).reshape(2, 128).T.copy(),
            "bk": bk[sl].reshape(2, 128).T.copy(),
            "bv": np.broadcast_to(bv[sl], (128, HS)).copy(),
            "ms": ms,
            "ob": ones_bf,
        })

    res = run_bass_kernel_spmd(nc, in_maps, core_ids=list(range(N_CORES)),
                               **_CACHE.get("run_kwargs", {}))
    _CACHE["last_result"] = res

    y = np.zeros((B, T, C), np.float32)
    for core in range(N_CORES):
        b = core // 4
        y[b] += res.results[core]["y"]
    y += bo
    return y
